# revision 8
# baseline (speedup 1.0000x reference)
"""Distributed statevector Hadamard-gate kernel for 8 TRN2 NeuronCores.

Problem: y = U @ x where U = kron_{i=0..23}(M if i in (0,5,10,15,20) else I2),
x is a 2^24-amplitude complex64 statevector (qudit 0 = most significant axis),
M is the 2x2 Hadamard (real-valued).

Strategy
--------
M is real, so real/imag parts transform independently -> treat x as a float
stream (interleaved re,im; bit-strides of qubit axes double).

Shard across 8 cores by qubits (1,2,3) (non-gate axes) -> every gate is local
to a core; no collectives.  The rel-err budget is 2e-2, so the wire format is
fp16 (host casts fp32->fp16 on the way in and fp16->fp32 on the way out):
halves HBM traffic, doubles DVE throughput (2x_1P mode), and quadruples PE
throughput vs fp32.  Expected end-to-end rel err ~6e-4.

Per core: a 2^22-fp16 slab whose bit layout is (MSB..LSB)

  q0 q4 q5 q6 q7 q8 | q9 q10 q11 q12 | q13 q14 q15 | q16..q23,reim (512-run)

On-chip layout: partition index p = (q0 q4 q5 q6 q7 q8)*2 + q10.  Gates on
q0,q5,q10 become ONE 128x128 fp16 matmul with a host-precomputed kron matrix
L (entries +-s^3*s^2, the scale of the two unnormalized DVE butterflies
folded in).  q15 and q20 are free-axis add/sub butterflies on the vector
engine (fp16 -> 2x mode).  PSUM (fp32) is evacuated by the scalar engine
with a cast to fp16.  Single HBM pass: DMA-in -> DVE bfly x2 -> PE matmul ->
ACT copy/cast -> DMA-out, pipelined over 8 x 1MB chunks per core.
"""

import math
import sys
import types

import numpy as np

import concourse.bass as bass
import concourse.mybir as mybir
from concourse.tile import TileContext
from concourse.bass_utils import run_bass_kernel_spmd


def _ensure_axon_hooks():
    """bass_utils' trace path does `from antenv.axon_hooks import ...`
    unconditionally; some images ship an `antenv` without that submodule,
    which would crash tracing.  Synthesize it (and register the ctypes NTFF
    hook when available) so tracing degrades gracefully instead.
    """
    try:
        import antenv.axon_hooks  # noqa: F401

        return
    except ImportError:
        pass
    try:
        import antenv
    except ImportError:
        return
    mod = types.ModuleType("antenv.axon_hooks")
    mod._hook = None

    def set_axon_ntff_profile_hook(hook):
        mod._hook = hook

    def get_axon_ntff_profile_hook():
        return mod._hook

    mod.set_axon_ntff_profile_hook = set_axon_ntff_profile_hook
    mod.get_axon_ntff_profile_hook = get_axon_ntff_profile_hook
    sys.modules["antenv.axon_hooks"] = mod
    antenv.axon_hooks = mod
    try:
        from trn_agent_boot.trn_boot import _ntff_profile_via_ctypes

        hook = _ntff_profile_via_ctypes("/opt/axon/libaxon_pjrt.so")
        if hook is not None:
            mod._hook = hook
    except Exception:
        pass


_ensure_axon_hooks()


def _legalize_waits(bir: dict) -> dict:
    """This image's walrus accepts only ONE sync-wait per TPB/DMA
    instruction; Tile emits up to ~4.  Hoist all but the last wait of each
    instruction into standalone EventSemaphore ops on the same engine,
    placed immediately before it — semantically identical (the engine
    blocks on them in program order).
    """
    for f in bir.get("functions", []):
        for b in f.get("blocks", []):
            out = []
            for i in b["instructions"]:
                si = i.get("sync_info") or {}
                waits = si.get("on_wait") or []
                if len(waits) > 1:
                    for k, wt in enumerate(waits[:-1]):
                        out.append({
                            "debug": i.get("debug", 0),
                            "engine": i["engine"],
                            "ins": [], "outs": [],
                            "name": f"hoistwait_{i['name']}_{k}",
                            "opcode": "EventSemaphore",
                            "sync_info": {"on_update": [], "on_wait": [wt]},
                        })
                    si["on_wait"] = [waits[-1]]
                out.append(i)
            b["instructions"] = out
    return bir


def _install_legalizer():
    import json as _json

    orig = bass.Bass.to_json_bytes
    if getattr(bass.Bass, "_wait_legalizer_installed", False):
        return

    def to_json_bytes(self, *a, **kw):
        raw = orig(self, *a, **kw)
        try:
            return _json.dumps(_legalize_waits(_json.loads(raw))).encode()
        except Exception:
            return raw

    bass.Bass.to_json_bytes = to_json_bytes
    bass.Bass._wait_legalizer_installed = True


_install_legalizer()

N_CORES = 8

_NC_CACHE: dict = {}

# set by kernel(): the BassKernelResults of the last run (exec_time_ns when
# run with BASS_TRACE=1) — used by the local test harness only
LAST_RESULT = None


def _build_nc(S: int, bfly):
    """Build the SPMD Bass program for one core.

    S: log2 of per-core slab fp16-element count (22 for complex64 input).
    bfly: ("had",) for add/sub butterfly (scale folded into L), or
          ("gen", a, b, c, d) for a generic real 2x2 gate on q15/q20.
    """
    RUN = 1 << (S - 13)       # contiguous run (q16..q23[,reim]): 512 / 256
    IN_FREE = 4 * RUN         # per-partition free elems of an in-chunk
    OUT_FREE = 8 * RUN        # out chunks pair up two in-chunks
    NOUT = 8                  # out-chunk bits: q9, q11, q12
    L_SUB = RUN // 32         # q21..q23[,reim] size below the q20 bit
    fp16 = mybir.dt.float16
    fp32 = mybir.dt.float32

    nc = bass.Bass()
    x = nc.declare_dram_parameter("x", [1 << S], fp16, isOutput=False)
    # four stationaries (W00 W01 W10 W11), Wij = M[i,j] * L, transposed
    w = nc.declare_dram_parameter("w", [128, 512], fp16, isOutput=False)
    y = nc.declare_dram_parameter("y", [1 << S], fp16, isOutput=True)

    # slab bits (MSB..LSB): P=(q0 q4 q5 q6 q7 q8), a=q9, t=q10, c=q11,
    # d=q12, e=q13, f = (q14 q15 run) contiguous 4*RUN.
    # Partition index p = P*2 + t.
    xv = x[:].rearrange(
        "(P a t c d e f) -> a c d e P t f",
        P=64, a=2, t=2, c=2, d=2, e=2, f=IN_FREE,
    )
    yv = y[:].rearrange(
        "(P a t c d f) -> a c d P t f",
        P=64, a=2, t=2, c=2, d=2, f=OUT_FREE,
    )

    with TileContext(nc) as tc:
        with (
            tc.tile_pool(name="wpool", bufs=1) as wpool,
            # one dedicated slot per chunk: in-DMAs never reuse a slot, so
            # they carry zero semaphore waits (walrus allows only one per
            # DMA pseudo-instruction)
            tc.tile_pool(name="inp", bufs=2 * NOUT) as inp,
            tc.tile_pool(name="b2p", bufs=3) as b2p,
            tc.tile_pool(name="outp", bufs=3) as outp,
            tc.tile_pool(name="psp", bufs=2, space="PSUM") as psp,
        ):
            wts = wpool.tile([128, 512], fp16, tag="wstage")
            nc.sync.dma_start(out=wts[:], in_=w[:])
            # stage via DVE so matmuls' weight dep is on the DVE semaphore
            wt = wpool.tile([128, 512], fp16, tag="wmain")
            nc.vector.tensor_copy(wt[:], wts[:])

            def wsl(i, j):  # stationary for output-bit i, input-bit j of q15
                k = 2 * i + j
                return wt[:, k * 128:(k + 1) * 128]

            # phase A: all in-DMAs up front — SP's HWDGE ring is FIFO, so
            # they stream back-to-back; outs are emitted after every in
            its = []
            for g in range(NOUT):
                gx = ((g >> 2) & 1, (g >> 1) & 1, g & 1)
                for e in range(2):
                    it = inp.tile([128, IN_FREE], fp16)
                    nc.sync.dma_start(out=it[:], in_=xv[gx + (e,)])
                    its.append(it)

            # phase B: compute + out-DMAs
            for g in range(NOUT):
                gx = ((g >> 2) & 1, (g >> 1) & 1, g & 1)
                ot = outp.tile([128, OUT_FREE], fp16)

                for e in range(2):
                    it = its[2 * g + e]

                    # q20 butterfly on DVE: free = (q14 q15 s, q20, low)
                    m2 = IN_FREE // (2 * L_SUB)
                    b2 = b2p.tile([128, IN_FREE], fp16)
                    jv = it[:].rearrange("p (m w l) -> p m w l", m=m2, w=2, l=L_SUB)
                    ov = b2[:].rearrange("p (m w l) -> p m w l", m=m2, w=2, l=L_SUB)
                    _bfly_pair(
                        nc, mybir, bfly,
                        ov[:, :, 0, :], ov[:, :, 1, :],
                        jv[:, :, 0, :], jv[:, :, 1, :],
                    )

                    # q15 gate folded into the matmul: PSUM-accumulate
                    # Wq15o0 @ b2[q15i=0] + Wq15o1 @ b2[q15i=1].
                    # psum layout: (q15o, q14, 512); ot needs (q14, q15o, 512)
                    ps = psp.tile([128, 2 * IN_FREE // 2], fp32)  # (128, 2048)
                    for q15o in range(2):
                        for q14h in range(2):
                            half = ps[:, q15o * 1024 + q14h * 512:
                                      q15o * 1024 + (q14h + 1) * 512]
                            nc.tensor.matmul(
                                half, wsl(q15o, 0),
                                b2[:, q14h * 1024:q14h * 1024 + 512],
                                start=True, stop=False,
                            )
                            nc.tensor.matmul(
                                half, wsl(q15o, 1),
                                b2[:, q14h * 1024 + 512:q14h * 1024 + 1024],
                                start=False, stop=True,
                            )
                    # PSUM evacuation + fp32->fp16 cast on the scalar
                    # engine, permuting (q15o, q14) -> (q14, q15o)
                    pv = ps[:].rearrange("p (q a u) -> p q a u", q=2, a=2, u=512)
                    ovv = ot[:, e * IN_FREE:(e + 1) * IN_FREE].rearrange(
                        "p (a q u) -> p q a u", a=2, q=2, u=512
                    )
                    nc.scalar.copy(ovv, pv)

                # out-DMAs also on the SP ring: they are emitted after all
                # in-DMAs, so they never stall an in-DMA in the FIFO
                nc.sync.dma_start(out=yv[gx], in_=ot[:])
    return nc


def _bfly_pair(nc, mb, bfly, out0, out1, i0, i1):
    """Apply a 2x2 gate to the (i0, i1) pair of equally-shaped views."""
    if bfly[0] == "had":
        nc.vector.tensor_add(out0, i0, i1)
        nc.vector.tensor_sub(out1, i0, i1)
    else:
        _, ga, gb, gc, gd = bfly
        # out0 = ga*x0 + gb*x1 ; out1 = gc*x0 + gd*x1
        nc.vector.tensor_scalar_mul(out0, i0, float(ga))
        nc.vector.scalar_tensor_tensor(
            out0, i1, float(gb), out0, mb.AluOpType.mult, mb.AluOpType.add
        )
        nc.vector.tensor_scalar_mul(out1, i0, float(gc))
        nc.vector.scalar_tensor_tensor(
            out1, i1, float(gd), out1, mb.AluOpType.mult, mb.AluOpType.add
        )


def _get_nc(S: int, bfly):
    key = (S, bfly)
    if key not in _NC_CACHE:
        _NC_CACHE[key] = _build_nc(S, bfly)
    return _NC_CACHE[key]


def _build_L(Mr: np.ndarray, fold_scale: float) -> np.ndarray:
    """128x128 real matrix applying M on partition bits q0, q5, q10.

    Partition index p = q0*64 + q4*32 + q5*16 + q6*8 + q7*4 + q8*2 + q10.
    """
    I2 = np.eye(2, dtype=np.float64)
    L = np.array([[1.0]])
    for F in (Mr, I2, Mr, I2, I2, I2, Mr):  # q0, q4, q5, q6, q7, q8, q10
        L = np.kron(L, F)
    return L * fold_scale


def kernel(x: np.ndarray, M: np.ndarray) -> np.ndarray:
    x = np.asarray(x)
    M = np.asarray(M)
    n, batch = x.shape
    assert n == 1 << 24 and batch == 1, (n, batch)

    is_complex = np.iscomplexobj(x)
    if is_complex:
        xc = np.ascontiguousarray(x, dtype=np.complex64)
        xf = xc.reshape(-1).view(np.float32)
    else:
        xf = np.ascontiguousarray(x, dtype=np.float32).reshape(-1)
    xh = xf.astype(np.float16)  # wire format: fp16 (tolerance is 2e-2)
    F = xh.size
    S = int(round(math.log2(F))) - 3  # per-core slab = F/8 elems

    # gate matrix: must be (essentially) real
    Mc = np.asarray(M, dtype=np.complex128)
    assert np.abs(Mc.imag).max() <= 1e-5 * max(np.abs(Mc.real).max(), 1e-30), (
        "complex-valued M is not supported"
    )
    Mr = Mc.real.copy()

    s0 = Mr[0, 0]
    had_form = (
        abs(s0) > 0
        and abs(Mr[0, 1] - s0) <= 1e-6 * abs(s0)
        and abs(Mr[1, 0] - s0) <= 1e-6 * abs(s0)
        and abs(Mr[1, 1] + s0) <= 1e-6 * abs(s0)
    )
    if had_form:
        bfly = ("had",)
        L = _build_L(Mr, fold_scale=s0)  # one unnormalized butterfly (q20)
    else:
        bfly = ("gen", Mr[0, 0], Mr[0, 1], Mr[1, 0], Mr[1, 1])
        L = _build_L(Mr, fold_scale=1.0)
    # four stationaries for the q15 gate folded into the matmul:
    # wT[:, 128*(2i+j)] block = (M[i,j] * L)^T
    wT = np.empty((128, 512), dtype=np.float16)
    for i in range(2):
        for j in range(2):
            wT[:, 128 * (2 * i + j):128 * (2 * i + j + 1)] = (
                (Mr[i, j] * L).T.astype(np.float16)
            )
    wT = np.ascontiguousarray(wT)

    nc = _get_nc(S, bfly if bfly[0] == "had" else bfly)

    # shard by qubits (1,2,3): xh.reshape(2[q0], 8[q1q2q3], F/16)
    xs = xh.reshape(2, 8, F // 16)
    in_maps = [
        {"x": np.ascontiguousarray(xs[:, cid, :]).reshape(-1), "w": wT}
        for cid in range(N_CORES)
    ]
    res = run_bass_kernel_spmd(nc, in_maps, list(range(N_CORES)))
    global LAST_RESULT
    LAST_RESULT = res
    outs = res.results

    yf = np.empty(F, dtype=np.float32)
    ys = yf.reshape(2, 8, F // 16)
    for cid in range(N_CORES):
        ys[:, cid, :] = outs[cid]["y"].reshape(2, F // 16).astype(np.float32)

    if is_complex:
        return yf.view(np.complex64).reshape(n, batch)
    return yf.reshape(n, batch)
